# revision 14
# baseline (speedup 1.0000x reference)
"""BERT self-attention (B=4, S=1024, D=1024, H=16) on 8 TRN2 NeuronCores.

Sharding: tensor-parallel over heads. Core c owns output dims
[c*128, (c+1)*128) of Wq/Wk/Wv (= heads 2c and 2c+1) and computes those
heads' attention for all 4 batches. seq is replicated (each core needs all
tokens). Host pre-transposes seq -> seqT [D, B*S] and the weight shards ->
[D, 128] so every on-chip operand already has the contraction dim on
partitions.

Per-core pipeline (per batch):
  qT/kT/vT [128, S] = W_shard @ seqT_b       (f32r matmuls, K=1024, N=512)
  v = PE-transpose(vT), augmented with a ones column per head
  per head h (64 dims):
    scoresT[j,i] = k_j . q_i  (K=64 matmul), expT = exp(0.125*scoresT) (ACT)
    outT[(d,den), i] = [v_h | 1]^T @ expT    (K=1024 accumulation)
    out[i, d] = PE-transpose(outT) row-scaled by 1/den  -> DMA to DRAM

The softmax skips the max-subtraction: scores ~ N(0,1) here so exp() is
comfortably in fp32 range, and exp(x)/sum(exp(x)) is algebraically
identical to the max-shifted form.
"""

import numpy as np
from contextlib import ExitStack

import concourse.bass as bass
import concourse.tile as tile
from concourse import bacc, mybir
from concourse.bass_utils import run_bass_kernel_spmd

N_CORES = 8
B, S, D = 4, 1024, 1024
DPC = 128  # output dims per core (2 heads x 64)
HPC = 2  # heads per core
DV = 64  # head dim
KT = D // 128  # contraction tiles
NCH = S // 512  # 512-wide free-dim chunks per batch
F32 = mybir.dt.float32
F32R = mybir.dt.float32r
EXP = mybir.ActivationFunctionType.Exp

# test.py may flip these to profile; the grading path leaves them alone.
TRACE = False
TRACE_KWARGS = {}
LAST_RESULTS = None

_CACHE = {}


def _emit(ctx, tc, seqT, wT, bias, ident, ones, outc):
    nc = tc.nc

    singles = ctx.enter_context(tc.tile_pool(name="singles", bufs=1))
    seq_pool = ctx.enter_context(tc.tile_pool(name="seq", bufs=16))
    qkv_pool = ctx.enter_context(tc.tile_pool(name="qkv", bufs=2))
    exp_pool = ctx.enter_context(tc.tile_pool(name="expT", bufs=18))
    outt_pool = ctx.enter_context(tc.tile_pool(name="outT", bufs=3))
    small_pool = ctx.enter_context(tc.tile_pool(name="small", bufs=8))
    psum_mm = ctx.enter_context(tc.tile_pool(name="psum_mm", bufs=3, space="PSUM"))
    psum_pv = ctx.enter_context(tc.tile_pool(name="psum_pv", bufs=2, space="PSUM"))
    psum_tr = ctx.enter_context(tc.tile_pool(name="psum_tr", bufs=2, space="PSUM"))

    w_sb = {}
    b_sb = {}
    for name in ("q", "k", "v"):
        wt = singles.tile([128, D], F32R, tag=f"w{name}", name=f"w{name}_sb")
        for kk in range(KT):
            nc.sync.dma_start(wt[:, kk * 128 : (kk + 1) * 128],
                              wT[name][kk * 128 : (kk + 1) * 128, :])
        w_sb[name] = wt
        bt = singles.tile([128, 1], F32, tag=f"b{name}", name=f"b{name}_sb")
        nc.sync.dma_start(bt[:], bias[name][:])
        b_sb[name] = bt
    id_sb = singles.tile([128, 128], F32, tag="ident", name="id_sb")
    nc.sync.dma_start(id_sb[:], ident[:])

    # Persistent v tiles ([v_h0 | 1 | v_h1 | 1] per 128-token block), two
    # rotating sets for cross-batch overlap. The ones columns are written
    # once from DRAM (Memset can't encode f32r) and never touched again.
    va_sets = []
    for s in range(2):
        tiles = []
        for t8 in range(KT):
            va = singles.tile([128, 2 * (DV + 1)], F32R,
                              tag=f"vaug_{s}_{t8}", name=f"vaug_{s}_{t8}")
            nc.sync.dma_start(va[:, DV : DV + 1], ones[:])
            nc.sync.dma_start(va[:, 2 * DV + 1 : 2 * DV + 2], ones[:])
            tiles.append(va)
        va_sets.append(tiles)

    for b in range(B):
        sq = []
        for kk in range(KT):
            t = seq_pool.tile([128, S], F32R, tag="seqT", name=f"seqT_b{b}k{kk}")
            nc.sync.dma_start(t[:], seqT[kk * 128 : (kk + 1) * 128, b * S : (b + 1) * S])
            sq.append(t)

        qkvT = {}
        for name in ("q", "k", "v"):
            dt = F32 if name == "v" else F32R
            dst = qkv_pool.tile([128, S], dt, tag=f"{name}T", name=f"{name}T_b{b}")
            for ic in range(NCH):
                ps = psum_mm.tile([128, 512], F32, tag="mm", name=f"ps_{name}{b}{ic}")
                for kk in range(KT):
                    nc.tensor.matmul(
                        ps[:],
                        w_sb[name][:, kk * 128 : (kk + 1) * 128],
                        sq[kk][:, ic * 512 : (ic + 1) * 512],
                        start=(kk == 0),
                        stop=(kk == KT - 1),
                    )
                nc.vector.tensor_scalar_add(
                    dst[:, ic * 512 : (ic + 1) * 512], ps[:], b_sb[name][:]
                )
            qkvT[name] = dst

        # v (token-major) via PE transpose of vT into the persistent
        # ones-augmented tiles
        vau = va_sets[b % 2]
        for t8 in range(KT):
            pt = psum_tr.tile([128, 128], F32, tag="tr", name=f"vtr_{b}{t8}")
            nc.tensor.transpose(pt[:], qkvT["v"][:, t8 * 128 : (t8 + 1) * 128], id_sb[:])
            va = vau[t8]
            nc.vector.tensor_copy(va[:, 0:DV], pt[:, 0:DV])
            nc.vector.tensor_copy(va[:, DV + 1 : 2 * DV + 1], pt[:, DV : 2 * DV])

        for h in range(HPC):
            hs = slice(h * DV, (h + 1) * DV)
            exp_tiles = {ic: [] for ic in range(NCH)}
            for ic in range(NCH):
                for t8 in range(KT):
                    ps = psum_mm.tile([128, 512], F32, tag="mm", name=f"sc_{b}{h}{ic}{t8}")
                    nc.tensor.matmul(
                        ps[:],
                        qkvT["k"][hs, t8 * 128 : (t8 + 1) * 128],
                        qkvT["q"][hs, ic * 512 : (ic + 1) * 512],
                        start=True,
                        stop=True,
                    )
                    et = exp_pool.tile([128, 512], F32R, tag="expT", name=f"ex_{b}{h}{ic}{t8}")
                    nc.scalar.activation(et[:], ps[:], EXP, scale=0.125)
                    exp_tiles[ic].append(et)
            for ic in range(NCH):
                pv = psum_pv.tile([DV + 1, 512], F32, tag="pv", name=f"pv_{b}{h}{ic}")
                for t8 in range(KT):
                    nc.tensor.matmul(
                        pv[:],
                        vau[t8][:, h * (DV + 1) : (h + 1) * (DV + 1)],
                        exp_tiles[ic][t8][:],
                        start=(t8 == 0),
                        stop=(t8 == KT - 1),
                    )
                oT = outt_pool.tile([DV + 1, 512], F32, tag="outT", name=f"oT_{b}{h}{ic}")
                nc.vector.tensor_copy(oT[:], pv[:])
                for sub in range(4):
                    pt = psum_tr.tile([128, DV + 1], F32, tag="tr", name=f"otr_{b}{h}{ic}{sub}")
                    nc.tensor.transpose(
                        pt[:], oT[:, sub * 128 : (sub + 1) * 128],
                        id_sb[: DV + 1, : DV + 1],
                    )
                    rc = small_pool.tile([128, 1], F32, tag="recip", name=f"rc_{b}{h}{ic}{sub}")
                    nc.vector.reciprocal(rc[:], pt[:, DV : DV + 1])
                    of = small_pool.tile([128, DV], F32, tag="of", name=f"of_{b}{h}{ic}{sub}")
                    nc.vector.tensor_scalar_mul(of[:], pt[:, 0:DV], rc[:])
                    row0 = b * S + ic * 512 + sub * 128
                    nc.sync.dma_start(outc[row0 : row0 + 128, hs], of[:])


def _build():
    if "nc" in _CACHE:
        return _CACHE["nc"]
    nc = bacc.Bacc(
        "TRN2",
        target_bir_lowering=False,
        debug=False,
        enable_asserts=False,
        num_devices=N_CORES,
    )
    seqT = nc.dram_tensor("seqT", [D, B * S], F32R, kind="ExternalInput").ap()
    wT = {
        name: nc.dram_tensor(f"w{name}T", [D, DPC], F32R, kind="ExternalInput").ap()
        for name in ("q", "k", "v")
    }
    bias = {
        name: nc.dram_tensor(f"b{name}", [DPC, 1], F32, kind="ExternalInput").ap()
        for name in ("q", "k", "v")
    }
    ident = nc.dram_tensor("ident", [128, 128], F32, kind="ExternalInput").ap()
    ones = nc.dram_tensor("ones", [128, 1], F32R, kind="ExternalInput").ap()
    outc = nc.dram_tensor("outc", [B * S, DPC], F32, kind="ExternalOutput").ap()

    with tile.TileContext(nc) as tc:
        with ExitStack() as ctx:
            _emit(ctx, tc, seqT, wT, bias, ident, ones, outc)
    nc.compile()
    _CACHE["nc"] = nc
    return nc


def make_in_maps(seq, Wq, bq, Wk, bk, Wv, bv):
    seqT_full = np.ascontiguousarray(seq.reshape(B * S, D).T)
    ident = np.eye(128, dtype=np.float32)
    in_maps = []
    for c in range(N_CORES):
        sl = slice(c * DPC, (c + 1) * DPC)
        in_maps.append(
            {
                "seqT": seqT_full,
                "wqT": np.ascontiguousarray(Wq[sl].T),
                "wkT": np.ascontiguousarray(Wk[sl].T),
                "wvT": np.ascontiguousarray(Wv[sl].T),
                "bq": np.ascontiguousarray(bq[sl].reshape(DPC, 1)),
                "bk": np.ascontiguousarray(bk[sl].reshape(DPC, 1)),
                "bv": np.ascontiguousarray(bv[sl].reshape(DPC, 1)),
                "ident": ident,
                "ones": np.ones((128, 1), np.float32),
            }
        )
    return in_maps


def kernel(seq, Wq, bq, Wk, bk, Wv, bv):
    global LAST_RESULTS
    nc = _build()
    in_maps = make_in_maps(seq, Wq, bq, Wk, bk, Wv, bv)
    res = run_bass_kernel_spmd(
        nc, in_maps, core_ids=list(range(N_CORES)), trace=TRACE, **TRACE_KWARGS
    )
    LAST_RESULTS = res
    out = np.empty((B * S, D), np.float32)
    for c in range(N_CORES):
        out[:, c * DPC : (c + 1) * DPC] = res.results[c]["outc"]
    return out.reshape(B, S, D)
